# revision 1
# baseline (speedup 1.0000x reference)
"""Conv2d(128->256, 3x3, pad=1) over a 256x256 image, sharded across 8 trn2 cores.

Strategy
--------
x: (C_in=128, H=256, W=256) f32, weight: (256, 128, 3, 3), bias: (256,1,1).
C_in == 128 maps exactly onto the SBUF partition (contraction) dim, so the
conv is 9 accumulated matmuls (one per kernel tap) per output tile:

    out[co, y, x] = sum_{ky,kx} W[ky,kx].T @ xpad[:, y+ky, x+kx]   + bias

Sharding: split H across the 8 cores (32 output rows each). Each core gets a
pre-padded slice xpad (128, 34, 258) with halo rows / zero borders prepared on
the host, so the device program is uniform SPMD. Per core, output tiles are
2 rows x 256 cols = 512 pixels (one fp32 PSUM bank); for each tile and each
128-channel output half we accumulate the taps, then VectorE applies
(psum * 1/512 + bias) on the PSUM->SBUF copy and the tile is DMA'd to DRAM
as fp16 (host upconverts; ~2^-11 rounding is far inside the error budget).

Mixed precision: 7 taps run as fp16 matmuls (full 16-bit PE rate, fp32 PSUM
accumulation). The last two taps (2,1),(2,2) run as ONE fp8e4m3 DoubleRow
matmul: the PE packs two fp8 weights per cell (virtual K=256), contracting
both taps in a single pass at ~1.1x the cost of one fp16 matmul instead of
two. Measured end-to-end rel err of this split on the reference distribution
is ~1.74e-2 (< the 2e-2 gate); pure fp8 would be ~3.4e-2 and fails.

All weights (fp16 and fp8) are pre-scaled by 512 on the host so the fp8
values land in e4m3's normal range (|w| <= 0.0295 would otherwise be ~50%
denormal, 20%+ quantization error); PSUM therefore holds 512*conv and the
bias copy fuses the 1/512 descale.

DMA: the two HWDGE queues (sync- and scalar-engine triggered) share the 16
DMA engines, so a concurrent bulk transfer steals wire bandwidth from the
critical path per-packet (and Tile's scheduler hoists dep-free dma_starts,
so a second queue cannot be held back). A single sync-engine queue therefore
carries ALL transfers in strict need-order. The fp8 (DoubleRow) matmuls of
tile t are deferred to after tile t+2's fp16 block so their data is never on
the critical path at all.
"""

import numpy as np
import ml_dtypes

import concourse.bass as bass
import concourse.tile as tile
from concourse import bacc, mybir
from concourse import bass_utils

N_CORES = 8
C_IN, C_OUT, KH, KW = 128, 256, 3, 3
H, W = 256, 256
H_S = H // N_CORES            # 32 output rows per core
HP, WP = H_S + 2, W + 2       # padded per-core input slice: 34 x 258
ROWS = 2                      # output rows per PSUM tile (N = ROWS*W = 512)
N_TILES = H_S // ROWS         # 16
N_HALF = C_OUT // 128         # 2 output-channel halves

F32 = mybir.dt.float32
F16 = mybir.dt.float16
F8 = mybir.dt.float8e4

# taps 0..6 = (0,0)..(2,0) in fp16; taps (2,1),(2,2) fused into one fp8
# DoubleRow matmul (see module docstring)
N_FP16_TAPS = 7
FP8_TAPS = [(2, 1), (2, 2)]
WSCALE = 512.0                # weight pre-scale; descaled in the bias copy

# x is split into row groups, each its own SBUF tile, so a group's matmuls
# can start as soon as its rows have landed (Tile deps are whole-tile). Each
# group covers GROUP_TILES[g] output tiles plus a 2-row halo overlap. The
# first group is 1 tile so the first matmuls start as early as possible.
# Few, large groups: every HWDGE trigger costs ~650ns of serial Sync-engine
# time and small transfers (small per-partition spans) get poor wire rates.
GROUP_TILES = [1, 2, 2, 11]
assert sum(GROUP_TILES) == N_TILES
N_GROUPS = len(GROUP_TILES)

# fp8 pre-shifted x: [128, 2, H_S*W] per core, split into 2 groups (tiles).
X8_GROUP_TILES = [3, 13]
DR_DEFER = 2                  # run tile t's fp8 matmuls after tile t+2's fp16

# dep-free dummy matmuls issued at program start: they run while the input
# DMAs are in flight and lift the PE clock gate (HAM) out of its cold 1.2 GHz
# state before the real matmul stream begins. Sized to bridge engine-boot
# (~6.6us) to first-data (~10us): ~2.8us at the cold rate, with short
# N=128 warmups at the end for fine granularity. Keeping the PE busy until
# real data arrives also avoids an idle gap that would reset the HAM ramp.
WARMUP_512 = 6
WARMUP_128 = 0

# Set by test harness: TRACE=True makes the next kernel() call capture an
# NTFF profile; the BassKernelResults lands in LAST_RESULT.
TRACE = False
TRACE_KW = {}
LAST_RESULT = None

_NC_CACHE = {}


def _build():
    nc = bacc.Bacc(
        "TRN2",
        target_bir_lowering=False,
        debug=False,
        enable_asserts=False,
        num_devices=N_CORES,
    )
    x_d = nc.dram_tensor("x", [C_IN, HP, WP], F16, kind="ExternalInput").ap()
    x8_d = nc.dram_tensor("x8", [C_IN, 2, H_S * W], F8, kind="ExternalInput").ap()
    # startup blob: row r = [xpad row r (258) | fp16 tap r weights (256)].
    # Fusing the first x rows and the first taps into ONE transfer doubles
    # the per-partition DMA span (faster early wire rate) and saves a
    # trigger — this transfer alone gates the first matmul.
    xw_d = nc.dram_tensor("xw", [C_IN, 3, WP + C_OUT], F16, kind="ExternalInput").ap()
    # fp16 taps 3..6 plus the fp32 bias folded in as 4 trailing fp16-typed
    # columns holding raw fp32 bytes (a separate bias tensor would be an
    # 8-byte-per-partition transfer — terrible packet size for the critical
    # DMA queue; and tensor_scalar requires an fp32 scalar2 operand)
    W_COLS = 4 * C_OUT + 2 * N_HALF
    w_d = nc.dram_tensor("w", [C_IN, W_COLS], F16, kind="ExternalInput").ap()
    w8_d = nc.dram_tensor("w8", [C_IN, 2, C_OUT], F8, kind="ExternalInput").ap()
    # output laid out [p, half, y, x] (channel h*128+p at [p, h]) so ONE
    # DMA per unit moves both halves: each dma_start trigger costs ~600ns of
    # serial engine time, and the last units' triggers gate the kernel tail
    o_d = nc.dram_tensor(
        "out", [128, N_HALF, H_S, W], F16, kind="ExternalOutput"
    ).ap()

    with tile.TileContext(nc) as tc:
        with (
            tc.tile_pool(name="xin", bufs=1) as xpool,
            tc.tile_pool(name="x8in", bufs=1) as x8pool,
            tc.tile_pool(name="wts", bufs=1) as wpool,
            tc.tile_pool(name="acc", bufs=8, space="PSUM") as ppool,
            tc.tile_pool(name="outs", bufs=6) as opool,
        ):
            # PE warmup: dep-free. The scratch operand is a raw (statically
            # allocated) SBUF tensor that is never written — its garbage
            # contents stream through the PE and land in a scratch PSUM bank
            # nobody reads.
            warm_sb = nc.alloc_sbuf_tensor("warm_src", [128, ROWS * W], F16).ap()
            warm_ps = ppool.tile([128, ROWS * W], F32, tag="ps", name="ps")
            for _ in range(WARMUP_512):
                nc.tensor.matmul(warm_ps[:], warm_sb[:, :128], warm_sb[:])
            for _ in range(WARMUP_128):
                nc.tensor.matmul(warm_ps[:, :128], warm_sb[:, :128], warm_sb[:, :128])
            # one dummy DoubleRow warmup: the first DR matmul pays a ~1us
            # perf-mode transition; absorb it here while DMAs are in flight
            warm8 = nc.alloc_sbuf_tensor("warm8", [128, 2, 512], F8).ap()
            warm_ps8 = ppool.tile([128, ROWS * W], F32, tag="ps", name="ps")
            nc.tensor.matmul(
                warm_ps8[:],
                warm8[:, :, :128],
                warm8[:],
                perf_mode=mybir.MatmulPerfMode.DoubleRow,
            )

            # separate tiles per transfer: Tile dependencies are whole-tile,
            # so early taps must not wait for the later tap transfers
            xw_sb = wpool.tile([128, 3, WP + C_OUT], F16, tag="xw", name="xw")
            wb_sb = wpool.tile([128, 2 * C_OUT], F16, tag="wb", name="wb")
            wc_sb = wpool.tile([128, W_COLS - 2 * C_OUT], F16, tag="wc", name="wc")
            w8_sb = wpool.tile([128, 2, C_OUT], F8, tag="w8", name="w8")
            group_rows = [gt * ROWS + 2 for gt in GROUP_TILES]
            group_rows[0] = 3  # group 0 x rows live in the xw blob / xg0b
            group_t0 = [sum(GROUP_TILES[:g]) for g in range(N_GROUPS)]
            x_groups = [
                xpool.tile([128, group_rows[g], WP], F16, tag=f"xg{g}", name=f"xg{g}")
                for g in range(N_GROUPS)
            ]
            xg0b = xpool.tile([128, 2, WP], F16, tag="xg0b", name="xg0b")
            x8_r0 = [0, X8_GROUP_TILES[0] * ROWS]
            x8_groups = [
                x8pool.tile(
                    [128, 2, gt * ROWS * W], F8, tag=f"x8g{g}", name=f"x8g{g}"
                )
                for g, gt in enumerate(X8_GROUP_TILES)
            ]

            # A single queue carries EVERYTHING in strict need-order: the
            # two HWDGE queues share the 16 DMA engines, so a "parallel"
            # second queue just steals wire bandwidth from the critical path
            # (and Tile's scheduler hoists dep-free dma_starts, so the
            # second queue cannot be held back). Need-order with the fp8
            # work deferred 2 tiles keeps every transfer ahead of its use.
            nc.sync.dma_start(xw_sb[:], xw_d[:])
            nc.sync.dma_start(wb_sb[:], w_d[:, : 2 * C_OUT])
            nc.sync.dma_start(wc_sb[:], w_d[:, 2 * C_OUT :])
            nc.sync.dma_start(xg0b[:], x_d[:, 2:4, :])
            def xg_dma(g):
                rg = group_t0[g] * ROWS
                nc.sync.dma_start(x_groups[g][:], x_d[:, rg : rg + group_rows[g], :])

            xg_dma(1)
            nc.sync.dma_start(w8_sb[:], w8_d[:])
            xg_dma(2)
            nc.sync.dma_start(
                x8_groups[0][:], x8_d[:, :, : X8_GROUP_TILES[0] * ROWS * W]
            )
            for g in range(3, N_GROUPS):
                xg_dma(g)
            nc.sync.dma_start(
                x8_groups[1][:], x8_d[:, :, X8_GROUP_TILES[0] * ROWS * W :]
            )

            def group_of_r0(r0):
                for g in reversed(range(N_GROUPS)):
                    if r0 >= group_t0[g] * ROWS:
                        return g
                raise AssertionError(r0)

            # processing units: 15 2-row tiles + two 1-row subtiles (the
            # split halves the bias-add + DMA latency off the final matmul)
            units = [(t * ROWS, ROWS) for t in range(N_TILES - 1)]
            units += [(H_S - 2, 1), (H_S - 1, 1)]
            live = {}

            def emit_fp16(u):
                r0, nrows = units[u]
                n = nrows * W
                g = group_of_r0(r0)
                yl = r0 - group_t0[g] * ROWS
                xg = x_groups[g]
                pss = [
                    ppool.tile([128, n], F32, tag="ps", name="ps")
                    for _ in range(N_HALF)
                ]
                live[u] = pss
                for k in range(N_FP16_TAPS):
                    ky, kx = divmod(k, KW)
                    if g == 0 and ky == 2:
                        rhs = xg0b[:, :nrows, kx : kx + W]
                    elif g == 0:
                        rhs = xw_sb[:, yl + ky : yl + ky + nrows, kx : kx + W]
                    else:
                        rhs = xg[:, yl + ky : yl + ky + nrows, kx : kx + W]
                    for h in range(N_HALF):
                        if k < 3:
                            c0 = WP + h * 128
                            lhsT = xw_sb[:, k, c0 : c0 + 128]
                        else:
                            wsb, kk = (wb_sb, k - 3) if k < 5 else (wc_sb, k - 5)
                            lhsT = wsb[
                                :, kk * C_OUT + h * 128 : kk * C_OUT + h * 128 + 128
                            ]
                        nc.tensor.matmul(
                            pss[h][:], lhsT, rhs, start=(k == 0), stop=False
                        )

            def emit_finish(u):
                r0, nrows = units[u]
                n = nrows * W
                g8 = 0 if r0 < x8_r0[1] else 1
                off8 = (r0 - x8_r0[g8]) * W
                pss = live.pop(u)
                for h in range(N_HALF):
                    # the two fp8 taps, contracted together in one DoubleRow
                    # pass (PE packs 2 fp8 weights per cell, virtual K=256)
                    nc.tensor.matmul(
                        pss[h][:],
                        w8_sb[:, :, h * 128 : h * 128 + 128],
                        x8_groups[g8][:, :, off8 : off8 + n],
                        start=False,
                        stop=True,
                        perf_mode=mybir.MatmulPerfMode.DoubleRow,
                    )
                b_ap = wc_sb[:, 2 * C_OUT :].bitcast(F32)  # fp32 bias bytes
                ot = opool.tile([128, N_HALF, n], F16, tag="ot", name="ot")
                # h0 on VectorE, h1 on ScalarE in parallel: the PSUM banks
                # free as fast as possible (the ring recycle gates tile t+3's
                # matmuls) and the halves land in one SBUF tile
                nc.vector.tensor_scalar(
                    out=ot[:, 0, :],
                    in0=pss[0][:],
                    scalar1=1.0 / WSCALE,
                    scalar2=b_ap[:, 0:1],
                    op0=mybir.AluOpType.mult,
                    op1=mybir.AluOpType.add,
                )
                nc.scalar.activation(
                    ot[:, 1, :],
                    pss[1][:],
                    mybir.ActivationFunctionType.Identity,
                    bias=b_ap[:, 1:2],
                    scale=1.0 / WSCALE,
                )
                # one combined-halves DMA per unit; alternate the trigger
                # between the two HWDGE engines so consecutive units' output
                # triggers (~600ns serial each) run in parallel at the tail
                eng = nc.sync if u % 2 == 0 else nc.scalar
                eng.dma_start(o_d[:, :, r0 : r0 + nrows, :], ot[:])

            # defer=2 in steady state; catch up BEFORE the last fp16 block
            # so only the final unit's finish (DR pair + bias + one DMA)
            # hangs off the end of the matmul stream
            n_u = len(units)
            for u in range(n_u):
                emit_fp16(u)
                if u == n_u - 2:
                    for f in range(u - DR_DEFER, u + 1):
                        emit_finish(f)
                elif u == n_u - 1:
                    emit_finish(u)
                elif u >= DR_DEFER:
                    emit_finish(u - DR_DEFER)
    nc.compile()
    return nc


def kernel(x, weight, bias):
    global LAST_RESULT
    if "nc" not in _NC_CACHE:
        _NC_CACHE["nc"] = _build()
    nc = _NC_CACHE["nc"]

    x = np.ascontiguousarray(np.asarray(x, dtype=np.float32))
    weight = np.asarray(weight, dtype=np.float32)
    bias = np.asarray(bias, dtype=np.float32)

    E4 = ml_dtypes.float8_e4m3

    # fp16 taps 0..6 (transposed to lhsT layout, pre-scaled) + bias columns
    wT = weight.transpose(1, 2, 3, 0).reshape(C_IN, KH * KW, C_OUT)
    # taps 3..6 tensor (taps 0-2 ride the startup blob with the first x rows)
    w16 = np.empty((C_IN, 4 * C_OUT + 2 * N_HALF), dtype=np.float16)
    w16[:, : 4 * C_OUT] = (wT[:, 3:N_FP16_TAPS, :] * WSCALE).reshape(C_IN, 4 * C_OUT)
    # b[p, h] = bias[h*128 + p] in fp32, folded into the weight tensor as
    # raw bytes in fp16-typed columns (device bitcasts back to fp32)
    bh = np.ascontiguousarray(bias.reshape(N_HALF, 128).T.astype(np.float32))
    w16[:, 4 * C_OUT :] = bh.view(np.float16)
    w012 = (wT[:, :3, :] * WSCALE).astype(np.float16)  # [C_IN, 3, C_OUT]

    # fp8 pair weights: w8[c, s, o] = e4m3(WSCALE * weight[o, c, tap_s])
    w8 = np.empty((C_IN, 2, C_OUT), dtype=E4)
    for s, (ky, kx) in enumerate(FP8_TAPS):
        w8[:, s, :] = (wT[:, ky * KW + kx, :] * WSCALE).astype(E4)

    # zero-padded fp16 image; per-core slices carry their halo rows
    xp = np.zeros((C_IN, H + 2, WP), dtype=np.float16)
    xp[:, 1 : H + 1, 1 : W + 1] = x.astype(np.float16)

    # fp8 image, quantized once, then pre-shifted per tap slot and cropped:
    # x8[c, s, y*W + x] = e4m3(xpad[c, y+2, x+1+s])  (taps (2,1),(2,2))
    x8full = np.zeros((C_IN, H + 2, WP), dtype=E4)
    x8full[:, 1 : H + 1, 1 : W + 1] = x.astype(E4)

    in_maps = []
    for c in range(N_CORES):
        y0 = c * H_S
        x8c = np.empty((C_IN, 2, H_S, W), dtype=E4)
        for s in range(2):
            x8c[:, s, :, :] = x8full[:, y0 + 2 : y0 + 2 + H_S, 1 + s : 1 + s + W]
        xw = np.empty((C_IN, 3, WP + C_OUT), dtype=np.float16)
        xw[:, :, :WP] = xp[:, y0 : y0 + 3, :]
        xw[:, :, WP:] = w012
        in_maps.append(
            {
                "x": np.ascontiguousarray(xp[:, y0 : y0 + HP, :]),
                "x8": np.ascontiguousarray(x8c.reshape(C_IN, 2, H_S * W)),
                "xw": xw,
                "w": w16,
                "w8": w8,
            }
        )

    kw = dict(TRACE_KW)
    if TRACE:
        kw.setdefault("trace", True)
        kw.setdefault("trace_cores", [0])
    res = bass_utils.run_bass_kernel_spmd(
        nc, in_maps, core_ids=list(range(N_CORES)), **kw
    )
    LAST_RESULT = res

    out = np.empty((C_OUT, H, W), dtype=np.float32)
    for c in range(N_CORES):
        # device layout [p, half, y, x] -> channel h*128+p
        arr = res.results[c]["out"].astype(np.float32)
        out[:, c * H_S : (c + 1) * H_S, :] = arr.transpose(1, 0, 2, 3).reshape(
            C_OUT, H_S, W
        )
    return out



# revision 12
# speedup vs baseline: 1.1350x; 1.1350x over previous
"""Conv2d(128->256, 3x3, pad=1) over a 256x256 image, sharded across 8 trn2 cores.

Strategy: 1-D Winograd F(2,3) along Y, all-fp16.
---------------------------------------------------
x: (C_in=128, H=256, W=256) f32, weight: (256, 128, 3, 3), bias: (256,1,1).
C_in == 128 maps exactly onto the SBUF partition (contraction) dim.

The 3x3 conv is decomposed as 3 kx taps x a 3-tap FIR in y. The y FIR is
computed with Winograd F(2,3): per output row PAIR (ty), the host builds 4
transformed row streams from padded input rows d_r = xpad[2ty + r]:

    v0 = d0 - d2,  v1 = d1 + d2,  v2 = d2 - d1,  v3 = d1 - d3

and 4 transformed weight sets per kx (g_r = w[:, :, ky=r, kx]):

    w~0 = g0, w~1 = (g0+g1+g2)/2, w~2 = (g0-g1+g2)/2, w~3 = -g2  (NEGATED)

On device, 4 plane matmuls per (sub-unit = 2 ty x one out-channel half),
each accumulating 3 kx taps (N=512):

    m_p = sum_kx  w~_p[kx].T @ v_p[ty0:ty0+2, kx:kx+W]

    out[2ty]   = m0 + m1 + m2 + bias           (even rows)
    out[2ty+1] = m1 - m2 + m3' + bias          (odd rows; m3' = -g2 conv)

That is 12 N=512 matmuls per 2ty-half where direct conv needs 18: a 33%
reduction in PE cycles, with NO fp8 anywhere (the old kernel ran 2 of 9
taps in fp8e4m3 at 1.74e-2 rel err; this is ~7e-4).

The inverse transform must combine PSUM planes post-PE (that sharing of
m1/m2 between even and odd rows IS the Winograd saving) and is budgeted
across the two fused-op engines (measured [128,512] op costs: ACT 687ns,
DVE-with-PSUM-operand 751ns, DVE 16-bit SBUF-only ~480ns):

  ScalarE: s1 = m1 + bias (fp16), s2p = +m2, s2n = -m2      3 ops ~2.0us
  VectorE: X = [m0|m3'] + bcast(s1)    (one op, stride-0 broadcast AP)
           OUT = X + [s2p|s2n] -> fp16 out rows             2 ops ~2.0us

against a 2.6us matmul span per sub-unit. PSUM planes pack pairwise into
[128, 2, 512] tiles ([m0|m3'], [m1|m2]) = 2 banks each; ring of 2
sub-units fills all 8 banks. Walrus rejects scalar_tensor_tensor with two
PSUM operands and ANY gpsimd tensor op, hence this exact split.

Sharding: H split across 8 cores (32 output rows = 16 ty pairs each); the
y halo is absorbed into the host transform (v rows never cross units).

DMA: one sync-engine queue carries all inputs in strict need-order. The
startup blob fuses v_ty0 + all half-0 weights + the fp32 bias (raw bytes
in fp16 cols) into ONE transfer: it alone gates the first real matmul.
Dep-free dummy matmuls bridge engine-boot (~6.6us) to first-data and lift
the PE HAM clock gate out of its cold 1.2 GHz state. Output tiles
([p, half, row-interleaved, x] so one DMA per 2ty unit moves both halves)
alternate sync/scalar trigger engines.
"""

import numpy as np

import concourse.bass as bass
import concourse.tile as tile
from concourse import bacc, mybir
from concourse import bass_utils

N_CORES = 8
C_IN, C_OUT, KH, KW = 128, 256, 3, 3
H, W = 256, 256
H_S = H // N_CORES            # 32 output rows per core
TY_S = H_S // 2               # 16 winograd row-pair units per core
WP = W + 2                    # padded row width: 258
N_HALF = C_OUT // 128         # 2 output-channel halves

F32 = mybir.dt.float32
F16 = mybir.dt.float16

# startup blob row p = [v_p ty0 (258) | h0 weights w~_p[kx0..2] (384) | 2 cols]
W0 = 3 * 128                  # per-plane weight cols in blob
BLOB_C = WP + W0 + 2          # 644
# units in ty pairs: first two 1-ty (fast start), last two 1-ty (short tail)
UNITS = [1, 1, 2, 2, 2, 2, 2, 2, 1, 1]
assert sum(UNITS) == TY_S
# v DMA groups for ty 1..15 (ty0 rides the blob)
V_GROUPS = [1, 2, 4, 8]
assert sum(V_GROUPS) == TY_S - 1

WARMUP_N512 = 7

# Set by test harness: TRACE=True makes the next kernel() call capture an
# NTFF profile; the BassKernelResults lands in LAST_RESULT.
TRACE = False
TRACE_KW = {}
LAST_RESULT = None

_NC_CACHE = {}


def _build():
    nc = bacc.Bacc(
        "TRN2",
        target_bir_lowering=False,
        debug=False,
        enable_asserts=False,
        num_devices=N_CORES,
    )
    vw_d = nc.dram_tensor("vw", [C_IN, 4, BLOB_C], F16, kind="ExternalInput").ap()
    wh1_d = nc.dram_tensor("wh1", [C_IN, 4, W0], F16, kind="ExternalInput").ap()
    v_d = nc.dram_tensor(
        "v", [C_IN, TY_S - 1, 4, WP], F16, kind="ExternalInput"
    ).ap()
    # output laid out [p, half, even/odd, ty, x] (channel h*128+p at [p, h];
    # row 2*ty+eo at [eo, ty]); host deinterleaves rows during the gather
    o_d = nc.dram_tensor(
        "out", [128, N_HALF, 2, TY_S, W], F16, kind="ExternalOutput"
    ).ap()

    with tile.TileContext(nc) as tc:
        with (
            tc.tile_pool(name="vin", bufs=1) as vpool,
            tc.tile_pool(name="wts", bufs=1) as wpool,
            tc.tile_pool(name="acc", bufs=4, space="PSUM") as ppool,
            tc.tile_pool(name="scr", bufs=6) as spool,
            tc.tile_pool(name="outs", bufs=4) as opool,
        ):
            # PE warmup: dep-free garbage through the PE into a dead tile
            warm_sb = nc.alloc_sbuf_tensor("warm_src", [128, 512], F16).ap()
            warm_ps = ppool.tile([128, 2, 512], F32, tag="ps", name="ps")
            for _ in range(WARMUP_N512):
                nc.tensor.matmul(warm_ps[:, 0, :], warm_sb[:, :128], warm_sb[:])

            blob_sb = wpool.tile([128, 4, BLOB_C], F16, tag="vw", name="vw")
            wh1_sb = wpool.tile([128, 4, W0], F16, tag="wh1", name="wh1")
            vg_sb = [
                vpool.tile([128, g, 4, WP], F16, tag=f"vg{i}", name=f"vg{i}")
                for i, g in enumerate(V_GROUPS)
            ]
            vg_t0 = [1 + sum(V_GROUPS[:i]) for i in range(len(V_GROUPS))]

            # single queue, strict need-order
            nc.sync.dma_start(blob_sb[:], vw_d[:])
            nc.sync.dma_start(wh1_sb[:], wh1_d[:])
            for i, g in enumerate(V_GROUPS):
                t0 = vg_t0[i] - 1
                nc.sync.dma_start(vg_sb[i][:], v_d[:, t0 : t0 + g, :, :])

            # fp32 bias bytes for half h live in blob row h cols 642:644
            bias_ap = [
                blob_sb[:, h, WP + W0 : WP + W0 + 2].bitcast(F32)
                for h in range(N_HALF)
            ]

            def v_ap(ty, nty, p, kx):
                """moving operand [128, nty, W] for ty..ty+nty-1, stream p"""
                if ty == 0:
                    assert nty == 1
                    return blob_sb[:, p, kx : kx + W]
                for i in reversed(range(len(V_GROUPS))):
                    if ty >= vg_t0[i]:
                        t = ty - vg_t0[i]
                        assert t + nty <= V_GROUPS[i]
                        return vg_sb[i][:, t : t + nty, p, kx : kx + W]
                raise AssertionError(ty)

            def w_ap(h, p, kx):
                if h == 0:
                    return blob_sb[:, p, WP + kx * 128 : WP + kx * 128 + 128]
                return wh1_sb[:, p, kx * 128 : kx * 128 + 128]

            def bcast2(ap):
                """[128, N] AP -> [128, 2, N] with a stride-0 middle dim"""
                return bass.AP(ap.tensor, ap.offset, [ap.ap[0], [0, 2], ap.ap[1]])

            add = mybir.AluOpType.add
            ty0 = 0
            for u, nty in enumerate(UNITS):
                n = nty * W
                for h in range(N_HALF):
                    # plane pairs: ta = [m1 | m2], tb = [m0 | m3']
                    ta = ppool.tile([128, 2, n], F32, tag="ps", name="ps")
                    tb = ppool.tile([128, 2, n], F32, tag="ps", name="ps")
                    for mt, planes in ((ta, (1, 2)), (tb, (0, 3))):
                        for j, p in enumerate(planes):
                            for kx in range(KW):
                                nc.tensor.matmul(
                                    mt[:, j, :],
                                    w_ap(h, p, kx),
                                    v_ap(ty0, nty, p, kx),
                                    start=(kx == 0),
                                    stop=(kx == KW - 1),
                                )
                    # ScalarE: stage m1+bias and +-m2 as fp16
                    s1 = spool.tile([128, n], F16, tag="s1", name="s1")
                    s2 = spool.tile([128, 2, n], F16, tag="s2", name="s2")
                    nc.scalar.activation(
                        s1[:],
                        ta[:, 0, :],
                        mybir.ActivationFunctionType.Identity,
                        bias=bias_ap[h][:, 0:1],
                        scale=1.0,
                    )
                    nc.scalar.activation(
                        s2[:, 0, :],
                        ta[:, 1, :],
                        mybir.ActivationFunctionType.Identity,
                        bias=0.0,
                        scale=1.0,
                    )
                    nc.scalar.activation(
                        s2[:, 1, :],
                        ta[:, 1, :],
                        mybir.ActivationFunctionType.Identity,
                        bias=0.0,
                        scale=-1.0,
                    )
                    # VectorE: X = [m0|m3'] + [s1|s1], OUT = X + [s2|-s2]
                    xp = spool.tile([128, 2, n], F16, tag="xp", name="xp")
                    nc.vector.scalar_tensor_tensor(
                        out=xp[:],
                        in0=tb[:],
                        scalar=0.0,
                        in1=bcast2(s1[:]),
                        op0=add,
                        op1=add,
                    )
                    # merged OUT writes [even-rows | odd-rows] contiguous;
                    # the per-half output DMA interleaves rows in DRAM
                    oh = opool.tile([128, 2, n], F16, tag="ot", name="ot")
                    nc.vector.scalar_tensor_tensor(
                        out=oh[:],
                        in0=xp[:],
                        scalar=0.0,
                        in1=s2[:],
                        op0=add,
                        op1=add,
                    )
                    nc.sync.dma_start(o_d[:, h, :, ty0 : ty0 + nty, :], oh[:])
                ty0 += nty
    nc.compile()
    return nc


def kernel(x, weight, bias):
    global LAST_RESULT
    if "nc" not in _NC_CACHE:
        _NC_CACHE["nc"] = _build()
    nc = _NC_CACHE["nc"]

    x = np.ascontiguousarray(np.asarray(x, dtype=np.float32))
    weight = np.asarray(weight, dtype=np.float32)
    bias = np.asarray(bias, dtype=np.float32)

    # zero-padded image; host computes the y-direction Winograd transform
    xpad = np.zeros((C_IN, H + 2, WP), dtype=np.float32)
    xpad[:, 1 : H + 1, 1 : W + 1] = x
    TY = H // 2
    r = 2 * np.arange(TY)
    v_full = np.empty((4, C_IN, TY, WP), dtype=np.float16)
    v_full[0] = xpad[:, r] - xpad[:, r + 2]
    v_full[1] = xpad[:, r + 1] + xpad[:, r + 2]
    v_full[2] = xpad[:, r + 2] - xpad[:, r + 1]
    v_full[3] = xpad[:, r + 1] - xpad[:, r + 3]

    # winograd weights per (p, kx): [c, o] fp16; w~3 NEGATED (odd row adds)
    wT = weight.transpose(1, 2, 3, 0)  # [c, ky, kx, o]
    wt = np.empty((4, KW, C_IN, C_OUT), dtype=np.float16)
    wt[0] = wT[:, 0].transpose(1, 0, 2)
    wt[1] = ((wT[:, 0] + wT[:, 1] + wT[:, 2]) / 2).transpose(1, 0, 2)
    wt[2] = ((wT[:, 0] - wT[:, 1] + wT[:, 2]) / 2).transpose(1, 0, 2)
    wt[3] = (-wT[:, 2]).transpose(1, 0, 2)

    # b[p, h] = bias[h*128 + p] in fp32, as raw bytes in fp16-typed cols
    bh = np.ascontiguousarray(bias.reshape(N_HALF, 128).T.astype(np.float32))
    bhv = bh.view(np.float16)  # [128, 4]

    wh1 = np.empty((C_IN, 4, W0), dtype=np.float16)
    for p in range(4):
        wh1[:, p, :] = (
            wt[p][:, :, 128:256].transpose(1, 0, 2).reshape(C_IN, KW * 128)
        )

    in_maps = []
    for c in range(N_CORES):
        t0 = c * TY_S
        blob = np.zeros((C_IN, 4, BLOB_C), dtype=np.float16)
        for p in range(4):
            blob[:, p, :WP] = v_full[p][:, t0, :]
            blob[:, p, WP : WP + W0] = (
                wt[p][:, :, 0:128].transpose(1, 0, 2).reshape(C_IN, KW * 128)
            )
        blob[:, 0, WP + W0 :] = bhv[:, 0:2]
        blob[:, 1, WP + W0 :] = bhv[:, 2:4]
        vc = np.ascontiguousarray(
            v_full[:, :, t0 + 1 : t0 + TY_S, :].transpose(1, 2, 0, 3)
        )
        in_maps.append({"vw": blob, "wh1": wh1, "v": vc})

    kw = dict(TRACE_KW)
    if TRACE:
        kw.setdefault("trace", True)
        kw.setdefault("trace_cores", [0])
    res = bass_utils.run_bass_kernel_spmd(
        nc, in_maps, core_ids=list(range(N_CORES)), **kw
    )
    LAST_RESULT = res

    out = np.empty((C_OUT, H, W), dtype=np.float32)
    for c in range(N_CORES):
        # device layout [p, half, eo, ty, x]: channel h*128+p, row 2*ty+eo
        arr = res.results[c]["out"].astype(np.float32)
        out[:, c * H_S : (c + 1) * H_S, :] = arr.transpose(1, 0, 3, 2, 4).reshape(
            C_OUT, H_S, W
        )
    return out
